# revision 1
# baseline (speedup 1.0000x reference)
"""Trainium2 Bass kernel for CausalSelfAttention (GQA + RoPE + QK-RMSNorm).

Sharding: 8 cores = DP(2 batches) x TP(4 head-groups).
Core c handles batch b=c//4, head group g=c%4 (q heads 4g..4g+3, kv head g).
Device: QKV proj (fp32r), RoPE+RMSNorm, causal attention (max-free softmax —
QK-norm bounds |score| <= sqrt(D)), PV in bf16 with a ones-column appended to
V so row-sums come out of the same matmul, per-512-column AllGather of the
transposed attention output across each 4-core group (overlapped with the
output projection), out-proj for this group's 512 output channels.
Host concatenates the 8 per-core [T, 512] results into [B, T, C].
"""

import sys
import numpy as np

for _p in ("/opt/trn_rl_repo", "/root/.axon_site/_ro/trn_rl_repo"):
    if _p not in sys.path:
        sys.path.append(_p)

import concourse.bass as bass
import concourse.mybir as mybir
import concourse.tile as tile
from concourse import bacc
from concourse.bass_utils import run_bass_kernel_spmd
from concourse.masks import make_identity

F32 = mybir.dt.float32
F32R = mybir.dt.float32r
BF16 = mybir.dt.bfloat16
AF = mybir.ActivationFunctionType
ALU = mybir.AluOpType

B, T, C = 2, 2048, 2048
H, KVH, D = 16, 4, 128
HLOC = H // 4          # q heads per core (TP=4)
DH = HLOC * D          # 512 output channels per core
EPS = 1.1920929e-07
NEG = -1.0e9           # additive causal mask value (exp underflows to 0)
N_CORES = 8

TT = 512               # t-tile (moving free dim)
# dev knobs (not used by the grading path)
CFG = {"skip_ag": False, "phases": 3, "trace_sim": False}
NKC = C // 128         # 16 contraction chunks for the projections
SM_SCALE = float(1.0 / np.sqrt(float(D)))


def r32(ap):
    return ap.bitcast(F32R)


def build_nc(t_seq=T, n_reps=1):
    """Build the SPMD program for one core (all cores run the same program).

    n_reps > 1 replicates the whole body for on-device timing (slope method).
    """
    nc = bacc.Bacc("TRN2", target_bir_lowering=False, debug=False,
                   num_devices=N_CORES)

    xT = nc.dram_tensor("xT", [C, t_seq], F32R, kind="ExternalInput").ap()
    wqT = nc.dram_tensor("wqT", [C, DH], F32R, kind="ExternalInput").ap()
    wkT = nc.dram_tensor("wkT", [C, D], F32R, kind="ExternalInput").ap()
    wvT = nc.dram_tensor("wvT", [C, D], F32R, kind="ExternalInput").ap()
    woT = nc.dram_tensor("woT", [C, DH], F32, kind="ExternalInput").ap()
    cos2 = nc.dram_tensor("cos2", [D, t_seq], F32, kind="ExternalInput").ap()
    sin2s = nc.dram_tensor("sin2s", [D, t_seq], F32, kind="ExternalInput").ap()
    out = nc.dram_tensor("out", [t_seq, DH], F32, kind="ExternalOutput").ap()

    groups = [[0, 1, 2, 3], [4, 5, 6, 7]]

    with tile.TileContext(nc, trace_sim=CFG["trace_sim"]) as tc:
        for _ in range(n_reps):
            build_body(tc, nc, xT, wqT, wkT, wvT, woT, cos2, sin2s, out,
                       groups, t_seq)
    nc.compile()
    return nc


def build_body(tc, nc, xT, wqT, wkT, wvT, woT, cos2, sin2s, out,
               groups, t_seq):
    from contextlib import ExitStack

    tt = TT
    nt = t_seq // tt       # t tiles
    nkb = t_seq // 128     # key blocks

    ctx = ExitStack()
    with ctx:
        # ---------- persistent pools ----------
        const_pool = ctx.enter_context(tc.tile_pool(name="const", bufs=1))
        qk_pool = ctx.enter_context(tc.tile_pool(name="qk", bufs=1))
        yv_pool = ctx.enter_context(tc.tile_pool(name="yv", bufs=1))
        dram = ctx.enter_context(tc.tile_pool(name="dram", bufs=1, space="DRAM"))

        ident = const_pool.tile([128, 128], F32, name="ident")
        make_identity(nc, ident[:])
        # swapmat: [[0, I64], [I64, 0]] — swaps the two D/2 halves via PE
        swapf = const_pool.tile([128, 128], F32, name="swapf")
        nc.gpsimd.memset(swapf[:], 0.0)
        for base in (64, -64):
            nc.gpsimd.affine_select(
                out=swapf[:], in_=swapf[:], compare_op=ALU.not_equal,
                fill=1.0, base=base, pattern=[[-1, 128]], channel_multiplier=1)
        swapmat = const_pool.tile([128, 128], F32R, name="swapmat")
        nc.scalar.activation(swapmat[:], swapf[:], AF.Copy)
        ones_f = const_pool.tile([128, 1], F32, name="ones_f")
        nc.gpsimd.memset(ones_f[:], 1.0)
        ones_col = const_pool.tile([128, 1], F32R, name="ones_col")
        nc.scalar.activation(ones_col[:], ones_f[:], AF.Copy)
        ones_row = const_pool.tile([1, 128], F32R, name="ones_row")
        nc.scalar.activation(ones_row[:], ones_f[0:1, :].to_broadcast([1, 128]),
                             AF.Copy)
        eps_t = const_pool.tile([1, 1], F32, name="eps_t")
        nc.gpsimd.memset(eps_t[:], EPS)
        smsc_f = const_pool.tile([1, 1], F32, name="smsc_f")
        nc.gpsimd.memset(smsc_f[:], SM_SCALE)


        # qT/kT normalized+roped, [D, t_seq] per head
        qTn = [qk_pool.tile([128, t_seq], F32R, name=f"qTn{h}") for h in range(HLOC)]
        kTn = qk_pool.tile([128, t_seq], F32R, name="kTn")
        # per-key-block exp scales: [128, 1] = SM_SCALE / rms(k)[tk]
        rks = [yv_pool.tile([128, 1], F32, name=f"rks{j}") for j in range(nkb)]
        # v_aug: per key block, [128 tk, 129] bf16 (col 128 = 1.0)
        v_aug = [yv_pool.tile([128, 129], BF16, name=f"vaug{j}") for j in range(nkb)]
        # attention output transposed: HLOC head-chunks of [128 c, t_seq]
        yT = [yv_pool.tile([128, t_seq], BF16, name=f"yT{h}") for h in range(HLOC)]

        # ================= phase 1: QKV projections =================
        with (
            tc.tile_pool(name="p1x", bufs=1) as p1x,
            tc.tile_pool(name="p1w", bufs=1) as p1w,
            tc.tile_pool(name="p1t", bufs=2) as p1t,
            tc.tile_pool(name="p1ps", bufs=4, space="PSUM") as p1ps,
            tc.tile_pool(name="p1sw", bufs=2, space="PSUM") as p1sw,
            tc.tile_pool(name="p1ss", bufs=1, space="PSUM") as p1ss,
        ):
            # weights (transposed, c-major) stay resident for phase 1.
            # Interleave x-chunk-0 and weight DMAs per c so the first
            # projection matmul can start after ~0.5 MB instead of ~10 MB.
            wq_sb = [p1w.tile([128, DH], F32R, name=f"wq{c}") for c in range(NKC)]
            wk_sb = [p1w.tile([128, D], F32R, name=f"wk{c}") for c in range(NKC)]
            wv_sb = [p1w.tile([128, D], F32R, name=f"wv{c}") for c in range(NKC)]
            cos_sb = p1w.tile([128, t_seq], F32, name="cos_sb")
            sin_sb = p1w.tile([128, t_seq], F32, name="sin_sb")
            nc.gpsimd.dma_start(cos_sb[:], cos2[:])
            nc.gpsimd.dma_start(sin_sb[:], sin2s[:])
            xt0 = []
            for c in range(NKC):
                xc = p1x.tile([128, tt], F32R, name=f"xt{c}", tag="xt",
                              bufs=NKC)
                nc.sync.dma_start(xc[:], xT[128 * c:128 * (c + 1), 0:tt])
                xt0.append(xc)
                nc.sync.dma_start(wq_sb[c][:], wqT[128 * c:128 * (c + 1), :])
            for c in range(NKC):
                nc.sync.dma_start(wk_sb[c][:], wkT[128 * c:128 * (c + 1), :])
                nc.sync.dma_start(wv_sb[c][:], wvT[128 * c:128 * (c + 1), :])
            vT = p1w.tile([128, t_seq], F32, name="vT")

            for i in range(nt):
                ts = slice(i * tt, (i + 1) * tt)
                # x^T chunk [C, tt] as NKC tiles of [128, tt]
                if i == 0:
                    xt = xt0
                else:
                    xt = []
                    for c in range(NKC):
                        xc = p1x.tile([128, tt], F32R, name=f"xt{c}", tag="xt",
                                      bufs=NKC)
                        nc.sync.dma_start(xc[:], xT[128 * c:128 * (c + 1), ts])
                        xt.append(xc)

                for h in range(HLOC):      # q heads: rope + rmsnorm
                    ps = p1ps.tile([128, tt], F32, name="qkv_ps")
                    for c in range(NKC):
                        nc.tensor.matmul(
                            ps[:], wq_sb[c][:, 128 * h:128 * (h + 1)],
                            xt[c][:], start=(c == 0), stop=(c == NKC - 1))
                    rope_norm(nc, p1t, p1sw, p1ss, ps,
                              cos_sb[:, ts], sin_sb[:, ts],
                              qTn[h][:, ts], swapmat, ones_col, ones_row, eps_t)
                # k head: rope, then 1/rms as per-tk exp scale (not applied
                # to kTn itself — folded into the softmax exp)
                ps = p1ps.tile([128, tt], F32, name="qkv_ps")
                for c in range(NKC):
                    nc.tensor.matmul(ps[:], wk_sb[c][:], xt[c][:],
                                     start=(c == 0), stop=(c == NKC - 1))
                rope_only(nc, p1t, p1sw, p1ss, ps,
                          cos_sb[:, ts], sin_sb[:, ts], kTn[:, ts],
                          swapmat, ones_col, eps_t, smsc_f,
                          [rks[j] for j in range(4 * i, min(4 * i + 4, nkb))],
                          ones_row)
                # v head (no rope/norm)
                ps = p1ps.tile([128, tt], F32, name="qkv_ps")
                for c in range(NKC):
                    nc.tensor.matmul(ps[:], wv_sb[c][:], xt[c][:],
                                     start=(c == 0), stop=(c == NKC - 1))
                nc.scalar.activation(vT[:, ts], ps[:], AF.Copy)

                # v_aug for this chunk: transpose to [tk, d], cast bf16
                for j in range(4 * i, min(4 * i + 4, nkb)):
                    tp = p1sw.tile([128, 128], F32, name="v_tp", tag="sw_ps",
                                   bufs=2)
                    nc.tensor.matmul(tp[:], vT[:, 128 * j:128 * (j + 1)],
                                     ident[:], is_transpose=True)
                    nc.gpsimd.memset(v_aug[j][:, 128:129], 1.0)
                    nc.scalar.activation(v_aug[j][:, 0:128], tp[:], AF.Copy)

        # ================= phase 2+3 shared SBUF =================
        with tc.tile_pool(name="p2m", bufs=1) as p2m:
            masks = []
            for r in range(4):
                m = p2m.tile([128, tt], BF16, name=f"mask{r}")
                nc.gpsimd.memset(m[:], 1.0)
                nc.gpsimd.affine_select(
                    out=m[:], in_=m[:], compare_op=ALU.is_ge, fill=0.0,
                    base=-128 * r, pattern=[[1, tt]], channel_multiplier=-1)
                masks.append(m)

            # wo tiles (DMA overlaps with attention)
            wo_sb = [p2m.tile([128, DH], BF16, name=f"wo{c}") for c in range(NKC)]
            for c in range(NKC):
                nc.gpsimd.dma_start(wo_sb[c][:], woT[128 * c:128 * (c + 1), :])

            ag_in = [dram.tile([DH, tt], BF16, name=f"ag_in{i}") for i in range(nt)]
            ag_out = [dram.tile([4 * DH, tt], BF16, name=f"ag_out{i}")
                      for i in range(nt)]

            # ---------- phase 2: causal attention ----------
            with (
                tc.tile_pool(name="p2pt", bufs=6) as p2pt,
                tc.tile_pool(name="p2on", bufs=6) as p2on,
                tc.tile_pool(name="p2st", bufs=2, space="PSUM") as p2st,
                tc.tile_pool(name="p2o", bufs=1, space="PSUM") as p2o,
                tc.tile_pool(name="p2tp", bufs=1, space="PSUM") as p2tp,
                tc.tile_pool(name="p3y", bufs=1) as p3y,
                tc.tile_pool(name="p3t", bufs=6) as p3t,
            ):
                for i in range(nt if CFG["phases"] >= 2 else 0):
                    nj = min(4 * i + 4, nkb)
                    for h in range(HLOC):
                        o_ps = [p2o.tile([128, 129], F32, name=f"o_ps{t}",
                                         tag=f"o{t}") for t in range(4)]
                        for j in range(nj):
                            r = j - 4 * i
                            # diagonal blocks r=1,2: the first 128*r score
                            # columns are fully masked — skip them (keep the
                            # moving dim >= 256 for full-rate fp32r)
                            off = 128 * r if r in (1, 2) else 0
                            npr = tt - off
                            st = p2st.tile([128, tt], F32, name="st_ps")
                            nc.tensor.matmul(
                                st[:, 0:npr], kTn[:, 128 * j:128 * (j + 1)],
                                qTn[h][:, i * tt + off:(i + 1) * tt])
                            pt = p2pt.tile([128, tt], BF16, name="pt")
                            nc.scalar.activation(pt[:, 0:npr], st[:, 0:npr],
                                                 AF.Exp, scale=rks[j][:])
                            if r >= 0:
                                nc.vector.tensor_mul(pt[:, 0:npr], pt[:, 0:npr],
                                                     masks[r][:, off:tt])
                            for t in range(4):
                                if j <= 4 * i + t:
                                    nc.tensor.matmul(
                                        o_ps[t][:],
                                        pt[:, 128 * t - off:128 * (t + 1) - off],
                                        v_aug[j][:], start=(j == 0),
                                        stop=(j == min(4 * i + t, nj - 1)))
                        for t in range(4):
                            rec = p2on.tile([128, 1], F32, name="rec")
                            nc.vector.reciprocal(rec[:], o_ps[t][:, 128:129])
                            o_n = p2on.tile([128, 128], F32, name="o_n")
                            nc.vector.tensor_scalar_mul(o_n[:],
                                                        o_ps[t][:, 0:128], rec[:])
                            tp = p2tp.tile([128, 128], F32, name="o_tp")
                            nc.tensor.matmul(tp[:], o_n[:], ident[:],
                                             is_transpose=True)
                            nc.vector.tensor_copy(
                                yT[h][:, i * tt + 128 * t:i * tt + 128 * (t + 1)],
                                tp[:])
                    # yT chunk i complete for all heads -> stage + AllGather
                    for h in range(HLOC):
                        nc.sync.dma_start(ag_in[i][128 * h:128 * (h + 1), :],
                                          yT[h][:, i * tt:(i + 1) * tt])
                    if not CFG["skip_ag"]:
                        nc.gpsimd.collective_compute(
                            "AllGather", ALU.bypass, replica_groups=groups,
                            ins=[ag_in[i][:]], outs=[ag_out[i][:]])

                # ---------- phase 3: output projection ----------
                for i in range(nt if CFG["phases"] >= 3 else 0):
                    yf = []
                    for c in range(NKC):
                        yc = p3y.tile([128, tt], BF16, name=f"yf{c}", tag="yf",
                                      bufs=2 * NKC)
                        nc.sync.dma_start(yc[:],
                                          ag_out[i][128 * c:128 * (c + 1), :])
                        yf.append(yc)
                    for t in range(4):
                        ps = p2o.tile([128, DH], F32, name="out_ps", tag="cp")
                        for c in range(NKC):
                            nc.tensor.matmul(
                                ps[:], yf[c][:, 128 * t:128 * (t + 1)],
                                wo_sb[c][:],
                                start=(c == 0), stop=(c == NKC - 1))
                        ot = p3t.tile([128, DH], F32, name="ot")
                        nc.vector.tensor_copy(ot[:], ps[:])
                        nc.sync.dma_start(
                            out[i * tt + 128 * t:i * tt + 128 * (t + 1), :],
                            ot[:])


def rope_only(nc, tmp_pool, sw_pool, ss_pool, ps, cos, sin_s, out_ap,
              swapmat, ones_col, eps_t, smsc, rks_blocks, ones_row):
    # (smsc is a plain-f32 [1,1] constant; the [1,128]x[1,1] transpose
    # matmuls run as fp32 — 1-row cost is negligible)
    """RoPE for k; writes roped (unnormalized) k to out_ap and the per-tk
    exp scales SM_SCALE/rms into rks_blocks ([128,1] each, via PE transpose
    of the [1, tt] reciprocal-rms row)."""
    ttl = ps.shape[-1]
    qf = tmp_pool.tile([128, ttl], F32R, name="qf", tag="qf")
    nc.scalar.activation(qf[:], ps[:], AF.Copy)
    sw = sw_pool.tile([128, ttl], F32, name="sw_ps")
    nc.tensor.matmul(sw[:], swapmat[:], qf[:])
    e1 = tmp_pool.tile([128, ttl], F32, name="e1", tag="e1")
    nc.gpsimd.tensor_mul(e1[:], qf[:], cos)
    qr = tmp_pool.tile([128, ttl], F32, name="qr", tag="qr")
    nc.vector.tensor_mul(qr[:], sw[:], sin_s)
    nc.gpsimd.tensor_add(out_ap, e1[:], qr[:])
    sq = tmp_pool.tile([128, ttl], F32R, name="sq", tag="sq")
    nc.scalar.activation(sq[:], out_ap, AF.Square)
    ss = ss_pool.tile([1, ttl], F32, name="ss_ps", tag="ss")
    nc.tensor.matmul(ss[:], ones_col[:], sq[:])
    sd = tmp_pool.tile([1, ttl], F32, name="sd", tag="sd")
    nc.scalar.activation(sd[:], ss[:], AF.Sqrt, scale=1.0 / 128.0,
                         bias=eps_t[:])
    rr = tmp_pool.tile([1, ttl], F32, name="rr", tag="rr")
    nc.vector.reciprocal(rr[:], sd[:])
    for b, rk in enumerate(rks_blocks):
        rkp = ss_pool.tile([128, 1], F32, name="rk_ps", tag="rb")
        nc.tensor.matmul(rkp[:], rr[0:1, 128 * b:128 * (b + 1)], smsc[:])
        nc.vector.tensor_copy(rk[:], rkp[:])


def rope_norm(nc, tmp_pool, sw_pool, ss_pool, ps, cos, sin_s, out_ap,
              swapmat, ones_col, ones_row, eps_t):
    """RoPE + RMS-norm. ps: [128 d, tt] PSUM (pre-rope head), out_ap: SBUF.

    cos is [cos; cos] (rows duplicated), sin_s is [sin; -sin], so
    rope = ps * cos + swap(ps) * sin_s with the half-swap done on PE.
    """
    ttl = ps.shape[-1]
    # f32r copy of the pre-rope head for the PE half-swap
    qf = tmp_pool.tile([128, ttl], F32R, name="qf", tag="qf")
    nc.scalar.activation(qf[:], ps[:], AF.Copy)
    sw = sw_pool.tile([128, ttl], F32, name="sw_ps")
    nc.tensor.matmul(sw[:], swapmat[:], qf[:])
    e1 = tmp_pool.tile([128, ttl], F32, name="e1", tag="e1")
    nc.gpsimd.tensor_mul(e1[:], qf[:], cos)
    qr = tmp_pool.tile([128, ttl], F32, name="qr", tag="qr")
    nc.vector.tensor_mul(qr[:], sw[:], sin_s)
    nc.gpsimd.tensor_add(qr[:], e1[:], qr[:])
    # sum of squares over d via PE (ones^T @ qr^2)
    sq = tmp_pool.tile([128, ttl], F32R, name="sq", tag="sq")
    nc.scalar.activation(sq[:], qr[:], AF.Square)
    ss = ss_pool.tile([1, ttl], F32, name="ss_ps", tag="ss")
    nc.tensor.matmul(ss[:], ones_col[:], sq[:])
    # rms = sqrt(ss/128 + eps); bcast to 128 partitions via PE; 1/rms on DVE
    sd = tmp_pool.tile([1, ttl], F32R, name="sd", tag="sd")
    nc.scalar.activation(sd[:], ss[:], AF.Sqrt, scale=1.0 / 128.0,
                         bias=eps_t[:])
    rb = ss_pool.tile([128, ttl], F32, name="rb_ps", tag="rb")
    nc.tensor.matmul(rb[:], ones_row[:], sd[:])
    rec = tmp_pool.tile([128, ttl], F32, name="rec", tag="rec")
    nc.vector.reciprocal(rec[:], rb[:])
    nc.vector.tensor_mul(out_ap, qr[:], rec[:])


_NC_CACHE = {}


def get_nc(t_seq=T, n_reps=1):
    key = (t_seq, n_reps)
    if key not in _NC_CACHE:
        _NC_CACHE[key] = build_nc(t_seq, n_reps)
    return _NC_CACHE[key]


def make_in_maps(x, cos, sin, Wq, Wk, Wv, Wo, t_seq=T):
    half = D // 2
    cosT = np.ascontiguousarray(cos.reshape(t_seq, half).T.astype(np.float32))
    sinT = np.ascontiguousarray(sin.reshape(t_seq, half).T.astype(np.float32))
    cos2 = np.concatenate([cosT, cosT], axis=0)
    sin2s = np.concatenate([sinT, -sinT], axis=0)
    wqTs, wkTs, wvTs, woTs = [], [], [], []
    for g in range(4):
        wqTs.append(np.ascontiguousarray(Wq[DH * g:DH * (g + 1), :].T))
        wkTs.append(np.ascontiguousarray(Wk[D * g:D * (g + 1), :].T))
        wvTs.append(np.ascontiguousarray(Wv[D * g:D * (g + 1), :].T))
        woTs.append(np.ascontiguousarray(Wo[DH * g:DH * (g + 1), :].T))
    xTs = [np.ascontiguousarray(x[b].T) for b in range(x.shape[0])]
    in_maps = []
    for c in range(N_CORES):
        b, g = c // 4, c % 4
        in_maps.append({
            "xT": xTs[b], "wqT": wqTs[g], "wkT": wkTs[g], "wvT": wvTs[g],
            "woT": woTs[g], "cos2": cos2, "sin2s": sin2s,
        })
    return in_maps


def kernel(x, cos, sin, Wq, Wk, Wv, Wo):
    x = np.asarray(x, dtype=np.float32)
    nc = get_nc(T)
    in_maps = make_in_maps(x, np.asarray(cos), np.asarray(sin),
                           np.asarray(Wq), np.asarray(Wk), np.asarray(Wv),
                           np.asarray(Wo), T)
    res = run_bass_kernel_spmd(nc, in_maps, core_ids=list(range(N_CORES)))
    outa = np.empty((B, T, C), dtype=np.float32)
    for c in range(N_CORES):
        b, g = c // 4, c % 4
        outa[b, :, DH * g:DH * (g + 1)] = res.results[c]["out"]
    return outa

